# revision 8
# baseline (speedup 1.0000x reference)
"""Trainium2 Bass kernel for nn_MultiHeadAttention (B=2, S=2048, D=1024, H=16).

Sharding: 8 cores = 2 batches x 4 head-groups (4 heads each).
Each core receives host-transposed activations x^T (layout [D, S]) for its
batch plus its head-group's slices of the projection weights, computes
  Q^T,K^T = W^T x^T   (per-head [DK, S], heads stacked on partitions)
  V       = x W       (natural [S, DK] per head, +ones column for softmax sums)
  scores^T[kv,q] = K Q^T / sqrt(DK), causal, exp (no max-sub needed: |s|<~4)
  out_av^T = V_aug^T attn^T  (row DK = softmax denominators)
  scale by gate/denominator, project with Wo rows -> partial output [S, D]
Host sums the 4 head-group partials per batch and adds bo.
"""

import os
import numpy as np

P = 128
CHUNK = 512  # q-chunk / matmul moving free dim

_BUILD_CACHE = {}


def _build(S, D, DOUT, HPC, DK, causal, debug=False):
    """Emit the Bass program (same program for all cores; data differs)."""
    import concourse.bass as bass
    import concourse.mybir as mybir
    import concourse.tile as tile
    from concourse import bacc
    from concourse.bass import ds, ts

    fp32 = mybir.dt.float32
    KC = D // P             # contraction k-chunks for projections
    GCOLS = HPC * DK        # this core's projection output width
    MT = GCOLS // P         # head-pair tiles (2 heads of DK=64 per tile)
    NCH = S // CHUNK        # q-chunks
    TPC = CHUNK // P        # kv tiles per q-chunk (4)
    NKV = S // P            # kv tiles total
    KC2 = GCOLS // P        # out-proj contraction chunks
    NOC = DOUT // CHUNK     # out-proj N chunks
    ST = S // P             # s-tiles
    assert DK * 2 == P and GCOLS % P == 0

    Act = mybir.ActivationFunctionType
    nc = bacc.Bacc()

    xqT = nc.declare_dram_parameter("xqT", [D, S], fp32, isOutput=False)
    xkT = nc.declare_dram_parameter("xkT", [D, S], fp32, isOutput=False)
    xvT = nc.declare_dram_parameter("xvT", [D, S], fp32, isOutput=False)
    wq_d = nc.declare_dram_parameter("wq", [D, GCOLS], fp32, isOutput=False)
    wk_d = nc.declare_dram_parameter("wk", [D, GCOLS], fp32, isOutput=False)
    wv_d = nc.declare_dram_parameter("wv", [D, GCOLS], fp32, isOutput=False)
    wo_d = nc.declare_dram_parameter("wo", [GCOLS, DOUT], fp32, isOutput=False)
    bq_d = nc.declare_dram_parameter("bq", [GCOLS], fp32, isOutput=False)
    bk_d = nc.declare_dram_parameter("bk", [GCOLS], fp32, isOutput=False)
    bv_d = nc.declare_dram_parameter("bv", [1, GCOLS], fp32, isOutput=False)
    wgq_d = nc.declare_dram_parameter("wgq", [D, HPC], fp32, isOutput=False)
    wgk_d = nc.declare_dram_parameter("wgk", [D, HPC], fp32, isOutput=False)
    bg_d = nc.declare_dram_parameter("bg", [1, HPC], fp32, isOutput=False)
    mtri_d = nc.declare_dram_parameter("mtri", [P, P], fp32, isOutput=False)
    outp = nc.declare_dram_parameter("out", [S, DOUT], fp32, isOutput=True)
    if debug:
        qt_d = nc.declare_dram_parameter("qt_dbg", [P, MT, S], fp32, isOutput=True)
        kt_d = nc.declare_dram_parameter("kt_dbg", [P, MT, S], fp32, isOutput=True)
        va_d = nc.declare_dram_parameter("va_dbg", [P, ST, HPC, DK + 1], fp32, isOutput=True)
        g_d = nc.declare_dram_parameter("g_dbg", [P, HPC], fp32, isOutput=True)
        hc_d = nc.declare_dram_parameter("hc_dbg", [P, KC2, S], fp32, isOutput=True)

    scale = 1.0 / float(np.sqrt(DK))

    with tile.TileContext(nc) as tc:
        with (
            tc.tile_pool(name="persist", bufs=1) as pp,
            tc.tile_pool(name="wts", bufs=1) as wp,
        ):
            qt = pp.tile([P, MT, S], fp32, tag="qt")
            kt = pp.tile([P, MT, S], fp32, tag="kt")
            vaug = pp.tile([P, ST, HPC, DK + 1], fp32, tag="vaug")
            hcat = pp.tile([P, KC2, S], fp32, tag="hcat")
            ones = pp.tile([P, P], fp32, tag="ones")
            nc.any.memset(ones[:], 1.0)
            nc.any.memset(vaug[:, :, :, DK : DK + 1], 1.0)

            wq = wp.tile([P, KC, GCOLS], fp32, tag="wq")
            wk = wp.tile([P, KC, GCOLS], fp32, tag="wk")
            wv = wp.tile([P, KC, GCOLS], fp32, tag="wv")
            wo = wp.tile([P, KC2, DOUT], fp32, tag="wo")
            nc.sync.dma_start(wq[:], wq_d.rearrange("(c p) n -> p c n", p=P))
            nc.sync.dma_start(wk[:], wk_d.rearrange("(c p) n -> p c n", p=P))
            nc.sync.dma_start(wv[:], wv_d.rearrange("(c p) n -> p c n", p=P))
            nc.sync.dma_start(wo[:], wo_d.rearrange("(c p) n -> p c n", p=P))
            wgq = wp.tile([P, KC, HPC], fp32, tag="wgq")
            wgk = wp.tile([P, KC, HPC], fp32, tag="wgk")
            nc.sync.dma_start(wgq[:], wgq_d.rearrange("(c p) h -> p c h", p=P))
            nc.sync.dma_start(wgk[:], wgk_d.rearrange("(c p) h -> p c h", p=P))
            bq = wp.tile([P, MT], fp32, tag="bq")
            bk = wp.tile([P, MT], fp32, tag="bk")
            nc.sync.dma_start(bq[:], bq_d.rearrange("(m p) -> p m", p=P))
            nc.sync.dma_start(bk[:], bk_d.rearrange("(m p) -> p m", p=P))
            bv = wp.tile([1, GCOLS], fp32, tag="bv")
            nc.sync.dma_start(bv[:], bv_d[:])
            bg = wp.tile([1, HPC], fp32, tag="bg")
            nc.sync.dma_start(bg[:], bg_d[:])
            mtri = wp.tile([P, P], fp32, tag="mtri")
            nc.sync.dma_start(mtri[:], mtri_d[:])
            gate64 = pp.tile([P, HPC], fp32, tag="gate64")
            lng64 = pp.tile([P, HPC], fp32, tag="lng64")

            # ---------------- Stage A: projections + pooled means + gate
            with (
                tc.tile_pool(name="xsub", bufs=3) as xp,
                tc.tile_pool(name="psa", bufs=3, space="PSUM") as psa,
                tc.tile_pool(name="pmisc", bufs=2) as pm,
            ):
                pooled_nk_q = pm.tile([P, KC, NCH], fp32, tag="pnq")
                pooled_nk_k = pm.tile([P, KC, NCH], fp32, tag="pnk")
                pooled_q = pm.tile([P, KC], fp32, tag="pq")
                pooled_k = pm.tile([P, KC], fp32, tag="pk")

                def proj_T(x_d, w_sb, b_sb, out_sb, pooled_nk):
                    # out_sb[:, m, s] = (W^T x^T)[m-tile] + b  (per-head-pair tiles)
                    x_t = x_d.rearrange("(c p) s -> p c s", p=P)
                    for n in range(NCH):
                        nsl = ds(n * CHUNK, CHUNK)
                        xs = xp.tile([P, KC, CHUNK], fp32, tag="xsub",
                                     name="xsub", bufs=3)
                        nc.sync.dma_start(xs[:], x_t[:, :, nsl])
                        if pooled_nk is not None:
                            for k in range(KC):
                                nc.vector.tensor_reduce(
                                    pooled_nk[:, k, n : n + 1], xs[:, k, :],
                                    mybir.AxisListType.X, mybir.AluOpType.add)
                        for m in range(MT):
                            ps = psa.tile([P, CHUNK], fp32, tag="psa", bufs=3)
                            for k in range(KC):
                                nc.tensor.matmul(
                                    ps[:], w_sb[:, k, ts(m, P)], xs[:, k, :],
                                    start=(k == 0), stop=(k == KC - 1))
                            nc.scalar.activation(
                                out_sb[:, m, nsl], ps[:], Act.Identity,
                                bias=b_sb[:, m : m + 1], scale=1.0)

                proj_T(xqT, wq, bq, qt, pooled_nk_q)
                proj_T(xkT, wk, bk, kt, pooled_nk_k)

                # V natural: lhsT = x_v^T chunks (stationary), rhs = Wv
                xvt_t = xvT.rearrange("(c p) s -> p c s", p=P)
                for n in range(NCH):
                    xs = xp.tile([P, KC, CHUNK], fp32, tag="xsub",
                                 name="xsub", bufs=3)
                    nc.sync.dma_start(xs[:], xvt_t[:, :, ds(n * CHUNK, CHUNK)])
                    for st2 in range(TPC):
                        st = n * TPC + st2
                        ps = psa.tile([P, GCOLS], fp32, tag="psv", bufs=2)
                        for k in range(KC):
                            nc.tensor.matmul(
                                ps[:], xs[:, k, ts(st2, P)], wv[:, k, :],
                                start=(k == 0), stop=False)
                        nc.tensor.matmul(
                            ps[:], ones[0:1, 0:P], bv[:], start=False, stop=True)
                        nc.any.tensor_copy(
                            vaug[:, st, :, 0:DK],
                            ps.rearrange("p (h d) -> p h d", d=DK))

                # pooled means over S (weights pre-scaled by 1/S on host)
                nc.vector.tensor_reduce(pooled_q[:], pooled_nk_q[:],
                                        mybir.AxisListType.X, mybir.AluOpType.add)
                nc.vector.tensor_reduce(pooled_k[:], pooled_nk_k[:],
                                        mybir.AxisListType.X, mybir.AluOpType.add)

                # gate logits -> sigmoid -> move to partition DK, ln()
                psg = psa.tile([1, HPC], fp32, tag="psg", bufs=1)
                for k in range(KC):
                    nc.tensor.matmul(psg[:], pooled_q[:, k : k + 1], wgq[:, k, :],
                                     start=(k == 0), stop=False)
                for k in range(KC):
                    nc.tensor.matmul(psg[:], pooled_k[:, k : k + 1], wgk[:, k, :],
                                     start=False, stop=False)
                nc.tensor.matmul(psg[:], ones[0:1, 0:1], bg[:],
                                 start=False, stop=True)
                gate0 = pm.tile([1, HPC], fp32, tag="gate0")
                nc.scalar.activation(gate0[:], psg[:], Act.Sigmoid)
                nc.sync.dma_start(gate64[DK : DK + 1, :], gate0[0:1, :])
                nc.scalar.activation(lng64[DK : DK + 1, :], gate64[DK : DK + 1, :],
                                     Act.Ln)

            if debug:
                nc.sync.dma_start(qt_d[:], qt[:])
                nc.sync.dma_start(kt_d[:], kt[:])
                nc.sync.dma_start(va_d[:], vaug[:])
                nc.sync.dma_start(g_d[:], gate64[:])
            # ---------------- Stage B: attention per head-pair
            with (
                tc.tile_pool(name="attn", bufs=4) as ap_,
                tc.tile_pool(name="rows", bufs=3) as rp,
                tc.tile_pool(name="otmp", bufs=3) as op_,
                tc.tile_pool(name="pssc", bufs=4, space="PSUM") as pssc,
                tc.tile_pool(name="psav", bufs=2, space="PSUM") as psav,
                tc.tile_pool(name="psbc", bufs=2, space="PSUM") as psbc,
            ):
                for hp in range(MT):
                    for j in range(NCH):
                        nkv_j = min(TPC * (j + 1), NKV) if causal else NKV
                        pe = psav.tile([DK + 1, CHUNK], fp32, tag="av_e", bufs=1)
                        po = psav.tile([DK + 1, CHUNK], fp32, tag="av_o", bufs=1)
                        for i in range(nkv_j):
                            t = i - TPC * j
                            if causal and t >= 0:
                                Ni = CHUNK - P * t
                                qoff = j * CHUNK + P * t
                            else:
                                Ni = CHUNK
                                qoff = j * CHUNK
                            for half, pav in ((0, pe), (1, po)):
                                hsl = slice(half * DK, (half + 1) * DK)
                                ps = pssc.tile([P, CHUNK], fp32, name="sc",
                                               tag=f"sc{half}", bufs=2)
                                nc.tensor.matmul(
                                    ps[:, :Ni], kt[hsl, hp, ts(i, P)],
                                    qt[hsl, hp, ds(qoff, Ni)],
                                    start=True, stop=True)
                                at = ap_.tile([P, CHUNK], fp32, tag=f"at{half}")
                                nc.scalar.activation(at[:, :Ni], ps[:, :Ni],
                                                     Act.Exp, scale=scale)
                                if causal and t >= 0:
                                    nc.vector.tensor_mul(
                                        at[:, 0:P], at[:, 0:P], mtri[:])
                                nc.tensor.matmul(
                                    pav[:, ds(qoff - j * CHUNK, Ni)],
                                    vaug[:, i, 2 * hp + half, :], at[:, :Ni],
                                    start=(i == 0), stop=(i == nkv_j - 1))
                        # normalize + gate; write headcat^T
                        jsl = ds(j * CHUNK, CHUNK)
                        for half, pav in ((0, pe), (1, po)):
                            h = 2 * hp + half
                            lnr = rp.tile([P, CHUNK], fp32, tag="lnr")
                            rr = rp.tile([P, CHUNK], fp32, tag="rr")
                            nc.scalar.activation(lnr[DK : DK + 1, :],
                                                 pav[DK : DK + 1, :], Act.Ln)
                            nc.scalar.activation(
                                rr[DK : DK + 1, :], lnr[DK : DK + 1, :],
                                Act.Exp, scale=-1.0,
                                bias=lng64[DK : DK + 1, h : h + 1])
                            bcp = psbc.tile([DK, CHUNK], fp32, tag="bc",
                                            bufs=2)
                            nc.tensor.matmul(bcp[:], ones[DK : DK + 1, 0:DK],
                                             rr[DK : DK + 1, :],
                                             start=True, stop=True)
                            bc = rp.tile([DK, CHUNK], fp32, tag="bcs")
                            nc.scalar.copy(bc[:], bcp[:])
                            if half == 0:
                                nc.vector.tensor_mul(hcat[0:DK, hp, jsl],
                                                     pav[0:DK, :], bc[:])
                            else:
                                ot = op_.tile([DK, CHUNK], fp32, tag="ot")
                                nc.vector.tensor_mul(ot[:], pav[0:DK, :], bc[:])
                                nc.sync.dma_start(hcat[DK:P, hp, jsl], ot[:])

            if debug:
                nc.sync.dma_start(hc_d[:], hcat[:])
            # ---------------- Stage C: output projection (partial; host adds bo)
            with (
                tc.tile_pool(name="osb", bufs=3) as ob,
                tc.tile_pool(name="psoc", bufs=3, space="PSUM") as psoc,
            ):
                for st in range(ST):
                    osb = ob.tile([P, DOUT], fp32, tag="osb")
                    for nh in range(NOC):
                        ps = psoc.tile([P, CHUNK], fp32, tag="pso", bufs=3)
                        for k2 in range(KC2):
                            nc.tensor.matmul(
                                ps[:], hcat[:, k2, ts(st, P)],
                                wo[:, k2, ds(nh * CHUNK, CHUNK)],
                                start=(k2 == 0), stop=(k2 == KC2 - 1))
                        nc.any.tensor_copy(osb[:, ds(nh * CHUNK, CHUNK)], ps[:])
                    nc.sync.dma_start(outp[ts(st, P), :], osb[:])

    nc.compile()
    return nc


def _prep_core_inputs(query, key_, value, Wq, bq, Wk, bk, Wv, bv, Wg, bg, Wo,
                      b, g, S, D, HPC, DK):
    GCOLS = HPC * DK
    H0 = g * HPC
    cs = slice(H0 * DK, H0 * DK + GCOLS)
    f32 = np.float32
    c = np.ascontiguousarray
    return {
        "xqT": c(query[b].T.astype(f32)),
        "xkT": c(key_[b].T.astype(f32)),
        "xvT": c(value[b].T.astype(f32)),
        "wq": c(Wq[:, cs].astype(f32)),
        "wk": c(Wk[:, cs].astype(f32)),
        "wv": c(Wv[:, cs].astype(f32)),
        "wo": c(Wo[cs, :].astype(f32)),
        "bq": c(bq[cs].astype(f32)),
        "bk": c(bk[cs].astype(f32)),
        "bv": c(bv[cs].astype(f32)[None, :]),
        "wgq": c((Wg[:D, H0 : H0 + HPC] / S).astype(f32)),
        "wgk": c((Wg[D:, H0 : H0 + HPC] / S).astype(f32)),
        "bg": c(bg[H0 : H0 + HPC].astype(f32)[None, :]),
        "mtri": np.triu(np.ones((P, P), f32)),
    }


_last_results = None


def kernel(query, key_, value, mask, Wq, bq, Wk, bk, Wv, bv, Wo, bo, Wg, bg):
    global _last_results
    from concourse.bass_utils import run_bass_kernel_spmd

    query = np.asarray(query)
    key_ = np.asarray(key_)
    value = np.asarray(value)
    mask = np.asarray(mask)
    B, S, D = query.shape
    H = np.asarray(bg).shape[0]
    DK = D // H
    DOUT = np.asarray(Wo).shape[1]
    NC_ = 8
    GROUPS = NC_ // B
    HPC = H // GROUPS

    causal = bool(
        np.array_equal(mask[0, 0], np.tril(np.ones((S, S), bool)))
    )
    if not causal:
        assert mask.all(), "only causal or all-true masks supported"

    key = (S, D, DOUT, HPC, DK, causal)
    if key not in _BUILD_CACHE:
        _BUILD_CACHE[key] = _build(*key)
    nc = _BUILD_CACHE[key]

    in_maps = []
    for c in range(NC_):
        b, gidx = divmod(c, GROUPS)
        in_maps.append(_prep_core_inputs(
            query, key_, value, Wq, bq, Wk, bk, Wv, bv, Wg, bg, Wo,
            b, gidx, S, D, HPC, DK))

    res = run_bass_kernel_spmd(nc, in_maps, core_ids=list(range(NC_)))
    _last_results = res

    out = np.zeros((B, S, DOUT), np.float32)
    for c in range(NC_):
        b = c // GROUPS
        out[b] += res.results[c]["out"]
    out += np.asarray(bo).astype(np.float32)
    return out


# revision 11
# speedup vs baseline: 2.0098x; 2.0098x over previous
"""Trainium2 Bass kernel for nn_MultiHeadAttention (B=2, S=2048, D=1024, H=16).

Sharding: 8 cores = 2 batches x 4 head-groups (4 heads each).
Each core receives host-transposed activations x^T (layout [D, S]) for its
batch plus its head-group's slices of the projection weights, computes
  Q^T,K^T = W^T x^T   (per-head [DK, S], heads stacked on partitions)
  V       = x W       (natural [S, DK] per head, +ones column for softmax sums)
  scores^T[kv,q] = K Q^T / sqrt(DK), causal, exp (no max-sub needed: |s|<~4)
  out_av^T = V_aug^T attn^T  (row DK = softmax denominators)
  scale by gate/denominator, project with Wo rows -> partial output [S, D]
Host sums the 4 head-group partials per batch and adds bo.
"""

import os
import numpy as np

P = 128
CHUNK = 512  # q-chunk / matmul moving free dim

_BUILD_CACHE = {}


def _build(S, D, DOUT, HPC, DK, causal, debug=False):
    """Emit the Bass program (same program for all cores; data differs)."""
    import concourse.bass as bass
    import concourse.mybir as mybir
    import concourse.tile as tile
    from concourse import bacc
    from concourse.bass import ds, ts

    fp32 = mybir.dt.float32
    bf16 = mybir.dt.bfloat16
    KC = D // P             # contraction k-chunks for projections
    GCOLS = HPC * DK        # this core's projection output width
    MT = GCOLS // P         # head-pair tiles (2 heads of DK=64 per tile)
    NCH = S // CHUNK        # q-chunks
    TPC = CHUNK // P        # kv tiles per q-chunk (4)
    NKV = S // P            # kv tiles total
    KC2 = GCOLS // P        # out-proj contraction chunks
    NOC = DOUT // CHUNK     # out-proj N chunks
    ST = S // P             # s-tiles
    assert DK * 2 == P and GCOLS % P == 0

    Act = mybir.ActivationFunctionType
    nc = bacc.Bacc()

    xqT = nc.declare_dram_parameter("xqT", [D, S], bf16, isOutput=False)
    xkT = nc.declare_dram_parameter("xkT", [D, S], bf16, isOutput=False)
    xvT = nc.declare_dram_parameter("xvT", [D, S], bf16, isOutput=False)
    wq_d = nc.declare_dram_parameter("wq", [D, GCOLS], bf16, isOutput=False)
    wk_d = nc.declare_dram_parameter("wk", [D, GCOLS], bf16, isOutput=False)
    wv_d = nc.declare_dram_parameter("wv", [D, GCOLS], bf16, isOutput=False)
    wo_d = nc.declare_dram_parameter("wo", [GCOLS, DOUT], bf16, isOutput=False)
    bq_d = nc.declare_dram_parameter("bq", [GCOLS], fp32, isOutput=False)
    bk_d = nc.declare_dram_parameter("bk", [GCOLS], fp32, isOutput=False)
    bv_d = nc.declare_dram_parameter("bv", [1, GCOLS], bf16, isOutput=False)
    wgq_d = nc.declare_dram_parameter("wgq", [D, HPC], fp32, isOutput=False)
    wgk_d = nc.declare_dram_parameter("wgk", [D, HPC], fp32, isOutput=False)
    bg_d = nc.declare_dram_parameter("bg", [1, HPC], fp32, isOutput=False)
    mtri_d = nc.declare_dram_parameter("mtri", [P, P], bf16, isOutput=False)
    outp = nc.declare_dram_parameter("out", [S, DOUT], fp32, isOutput=True)
    if debug:
        qt_d = nc.declare_dram_parameter("qt_dbg", [P, MT, S], fp32, isOutput=True)
        kt_d = nc.declare_dram_parameter("kt_dbg", [P, MT, S], fp32, isOutput=True)
        va_d = nc.declare_dram_parameter("va_dbg", [P, ST, HPC, DK + 1], fp32, isOutput=True)
        g_d = nc.declare_dram_parameter("g_dbg", [P, HPC], fp32, isOutput=True)
        hc_d = nc.declare_dram_parameter("hc_dbg", [P, KC2, S], fp32, isOutput=True)

    scale = 1.0 / float(np.sqrt(DK))

    with tile.TileContext(nc) as tc:
        with (
            tc.tile_pool(name="persist", bufs=1) as pp,
            tc.tile_pool(name="wts", bufs=1) as wp,
        ):
            qt = pp.tile([P, MT, S], bf16, tag="qt")
            kt = pp.tile([P, MT, S], bf16, tag="kt")
            vaug = pp.tile([P, ST, HPC, DK + 1], bf16, tag="vaug")
            hcat = pp.tile([P, KC2, S], bf16, tag="hcat")
            ones = pp.tile([P, P], fp32, tag="ones")
            nc.any.memset(ones[:], 1.0)
            ones_bf = pp.tile([1, P], bf16, tag="ones_bf")
            nc.any.memset(ones_bf[:], 1.0)
            nc.any.memset(vaug[:, :, :, DK : DK + 1], 1.0)

            wq = wp.tile([P, KC, GCOLS], bf16, tag="wq")
            wk = wp.tile([P, KC, GCOLS], bf16, tag="wk")
            wv = wp.tile([P, KC, GCOLS], bf16, tag="wv")
            wo = wp.tile([P, KC2, DOUT], bf16, tag="wo")
            nc.sync.dma_start(wq[:], wq_d.rearrange("(c p) n -> p c n", p=P))
            nc.sync.dma_start(wk[:], wk_d.rearrange("(c p) n -> p c n", p=P))
            nc.sync.dma_start(wv[:], wv_d.rearrange("(c p) n -> p c n", p=P))
            nc.sync.dma_start(wo[:], wo_d.rearrange("(c p) n -> p c n", p=P))
            wgq = wp.tile([P, KC, HPC], fp32, tag="wgq")
            wgk = wp.tile([P, KC, HPC], fp32, tag="wgk")
            nc.sync.dma_start(wgq[:], wgq_d.rearrange("(c p) h -> p c h", p=P))
            nc.sync.dma_start(wgk[:], wgk_d.rearrange("(c p) h -> p c h", p=P))
            bq = wp.tile([P, MT], fp32, tag="bq")
            bk = wp.tile([P, MT], fp32, tag="bk")
            nc.sync.dma_start(bq[:], bq_d.rearrange("(m p) -> p m", p=P))
            nc.sync.dma_start(bk[:], bk_d.rearrange("(m p) -> p m", p=P))
            bv = wp.tile([1, GCOLS], bf16, tag="bv")
            nc.sync.dma_start(bv[:], bv_d[:])
            bg = wp.tile([1, HPC], fp32, tag="bg")
            nc.sync.dma_start(bg[:], bg_d[:])
            mtri = wp.tile([P, P], bf16, tag="mtri")
            nc.sync.dma_start(mtri[:], mtri_d[:])
            gate64 = pp.tile([P, HPC], fp32, tag="gate64")
            lng64 = pp.tile([P, HPC], fp32, tag="lng64")

            # ---------------- Stage A: projections + pooled means + gate
            with (
                tc.tile_pool(name="xsub", bufs=3) as xp,
                tc.tile_pool(name="psa", bufs=3, space="PSUM") as psa,
                tc.tile_pool(name="pmisc", bufs=2) as pm,
            ):
                pooled_nk_q = pm.tile([P, KC, NCH], fp32, tag="pnq")
                pooled_nk_k = pm.tile([P, KC, NCH], fp32, tag="pnk")
                pooled_q = pm.tile([P, KC], fp32, tag="pq")
                pooled_k = pm.tile([P, KC], fp32, tag="pk")

                def proj_T(x_d, w_sb, b_sb, out_sb, pooled_nk):
                    # out_sb[:, m, s] = (W^T x^T)[m-tile] + b  (per-head-pair tiles)
                    x_t = x_d.rearrange("(c p) s -> p c s", p=P)
                    for n in range(NCH):
                        nsl = ds(n * CHUNK, CHUNK)
                        xs = xp.tile([P, KC, CHUNK], bf16, tag="xsub",
                                     name="xsub", bufs=3)
                        nc.sync.dma_start(xs[:], x_t[:, :, nsl])
                        if pooled_nk is not None:
                            for k in range(KC):
                                nc.vector.tensor_reduce(
                                    pooled_nk[:, k, n : n + 1], xs[:, k, :],
                                    mybir.AxisListType.X, mybir.AluOpType.add)
                        for m in range(MT):
                            ps = psa.tile([P, CHUNK], fp32, tag="psa", bufs=3)
                            for k in range(KC):
                                nc.tensor.matmul(
                                    ps[:], w_sb[:, k, ts(m, P)], xs[:, k, :],
                                    start=(k == 0), stop=(k == KC - 1))
                            nc.scalar.activation(
                                out_sb[:, m, nsl], ps[:], Act.Identity,
                                bias=b_sb[:, m : m + 1], scale=1.0)

                proj_T(xqT, wq, bq, qt, pooled_nk_q)
                proj_T(xkT, wk, bk, kt, pooled_nk_k)

                # V natural: lhsT = x_v^T chunks (stationary), rhs = Wv
                xvt_t = xvT.rearrange("(c p) s -> p c s", p=P)
                for n in range(NCH):
                    xs = xp.tile([P, KC, CHUNK], bf16, tag="xsub",
                                 name="xsub", bufs=3)
                    nc.sync.dma_start(xs[:], xvt_t[:, :, ds(n * CHUNK, CHUNK)])
                    for st2 in range(TPC):
                        st = n * TPC + st2
                        ps = psa.tile([P, GCOLS], fp32, tag="psv", bufs=2)
                        for k in range(KC):
                            nc.tensor.matmul(
                                ps[:], xs[:, k, ts(st2, P)], wv[:, k, :],
                                start=(k == 0), stop=False)
                        nc.tensor.matmul(
                            ps[:], ones_bf[0:1, 0:P], bv[:], start=False, stop=True)
                        nc.vector.tensor_copy(
                            vaug[:, st, :, 0:DK],
                            ps.rearrange("p (h d) -> p h d", d=DK))

                # pooled means over S (weights pre-scaled by 1/S on host)
                nc.vector.tensor_reduce(pooled_q[:], pooled_nk_q[:],
                                        mybir.AxisListType.X, mybir.AluOpType.add)
                nc.vector.tensor_reduce(pooled_k[:], pooled_nk_k[:],
                                        mybir.AxisListType.X, mybir.AluOpType.add)

                # gate logits -> sigmoid -> move to partition DK, ln()
                psg = psa.tile([1, HPC], fp32, tag="psg", bufs=1)
                for k in range(KC):
                    nc.tensor.matmul(psg[:], pooled_q[:, k : k + 1], wgq[:, k, :],
                                     start=(k == 0), stop=False)
                for k in range(KC):
                    nc.tensor.matmul(psg[:], pooled_k[:, k : k + 1], wgk[:, k, :],
                                     start=False, stop=False)
                nc.tensor.matmul(psg[:], ones[0:1, 0:1], bg[:],
                                 start=False, stop=True)
                gate0 = pm.tile([1, HPC], fp32, tag="gate0")
                nc.scalar.activation(gate0[:], psg[:], Act.Sigmoid)
                nc.sync.dma_start(gate64[DK : DK + 1, :], gate0[0:1, :])
                nc.scalar.activation(lng64[DK : DK + 1, :], gate64[DK : DK + 1, :],
                                     Act.Ln)

            if debug:
                nc.sync.dma_start(qt_d[:], qt[:])
                nc.sync.dma_start(kt_d[:], kt[:])
                nc.sync.dma_start(va_d[:], vaug[:])
                nc.sync.dma_start(g_d[:], gate64[:])
            # ---------------- Stage B: attention per head-pair
            with (
                tc.tile_pool(name="attn", bufs=4) as ap_,
                tc.tile_pool(name="rows", bufs=3) as rp,
                tc.tile_pool(name="otmp", bufs=3) as op_,
                tc.tile_pool(name="pssc", bufs=4, space="PSUM") as pssc,
                tc.tile_pool(name="psav", bufs=2, space="PSUM") as psav,
                tc.tile_pool(name="psbc", bufs=2, space="PSUM") as psbc,
            ):
                for hp in range(MT):
                    for j in range(NCH):
                        nkv_j = min(TPC * (j + 1), NKV) if causal else NKV
                        pe = psav.tile([DK + 1, CHUNK], fp32, tag="av_e", bufs=1)
                        po = psav.tile([DK + 1, CHUNK], fp32, tag="av_o", bufs=1)
                        for i in range(nkv_j):
                            t = i - TPC * j
                            if causal and t >= 0:
                                Ni = CHUNK - P * t
                                qoff = j * CHUNK + P * t
                            else:
                                Ni = CHUNK
                                qoff = j * CHUNK
                            for half, pav in ((0, pe), (1, po)):
                                hsl = slice(half * DK, (half + 1) * DK)
                                ps = pssc.tile([P, CHUNK], fp32, name="sc",
                                               tag=f"sc{half}", bufs=2)
                                nc.tensor.matmul(
                                    ps[:, :Ni], kt[hsl, hp, ts(i, P)],
                                    qt[hsl, hp, ds(qoff, Ni)],
                                    start=True, stop=True)
                                at = ap_.tile([P, CHUNK], bf16, tag=f"at{half}")
                                nc.scalar.activation(at[:, :Ni], ps[:, :Ni],
                                                     Act.Exp, scale=scale)
                                if causal and t >= 0:
                                    nc.vector.tensor_mul(
                                        at[:, 0:P], at[:, 0:P], mtri[:])
                                nc.tensor.matmul(
                                    pav[:, ds(qoff - j * CHUNK, Ni)],
                                    vaug[:, i, 2 * hp + half, :], at[:, :Ni],
                                    start=(i == 0), stop=(i == nkv_j - 1))
                        # normalize + gate; write headcat^T
                        jsl = ds(j * CHUNK, CHUNK)
                        for half, pav in ((0, pe), (1, po)):
                            h = 2 * hp + half
                            lnr = rp.tile([P, CHUNK], fp32, tag="lnr")
                            rr = rp.tile([P, CHUNK], fp32, tag="rr")
                            nc.scalar.activation(lnr[DK : DK + 1, :],
                                                 pav[DK : DK + 1, :], Act.Ln)
                            nc.scalar.activation(
                                rr[DK : DK + 1, :], lnr[DK : DK + 1, :],
                                Act.Exp, scale=-1.0,
                                bias=lng64[DK : DK + 1, h : h + 1])
                            bcp = psbc.tile([DK, CHUNK], fp32, tag="bc",
                                            bufs=2)
                            nc.tensor.matmul(bcp[:], ones[DK : DK + 1, 0:DK],
                                             rr[DK : DK + 1, :],
                                             start=True, stop=True)
                            bc = rp.tile([DK, CHUNK], fp32, tag="bcs")
                            nc.scalar.copy(bc[:], bcp[:])
                            if half == 0:
                                nc.vector.tensor_mul(hcat[0:DK, hp, jsl],
                                                     pav[0:DK, :], bc[:])
                            else:
                                ot = op_.tile([DK, CHUNK], bf16, tag="ot")
                                nc.vector.tensor_mul(ot[:], pav[0:DK, :], bc[:])
                                nc.sync.dma_start(hcat[DK:P, hp, jsl], ot[:])

            if debug:
                nc.sync.dma_start(hc_d[:], hcat[:])
            # ---------------- Stage C: output projection (partial; host adds bo)
            with (
                tc.tile_pool(name="osb", bufs=3) as ob,
                tc.tile_pool(name="psoc", bufs=3, space="PSUM") as psoc,
            ):
                for st in range(ST):
                    osb = ob.tile([P, DOUT], fp32, tag="osb")
                    for nh in range(NOC):
                        ps = psoc.tile([P, CHUNK], fp32, tag="pso", bufs=3)
                        for k2 in range(KC2):
                            nc.tensor.matmul(
                                ps[:], hcat[:, k2, ts(st, P)],
                                wo[:, k2, ds(nh * CHUNK, CHUNK)],
                                start=(k2 == 0), stop=(k2 == KC2 - 1))
                        nc.vector.tensor_copy(osb[:, ds(nh * CHUNK, CHUNK)], ps[:])
                    nc.sync.dma_start(outp[ts(st, P), :], osb[:])

    nc.compile()
    return nc


def _prep_core_inputs(query, key_, value, Wq, bq, Wk, bk, Wv, bv, Wg, bg, Wo,
                      b, g, S, D, HPC, DK):
    import ml_dtypes
    GCOLS = HPC * DK
    H0 = g * HPC
    cs = slice(H0 * DK, H0 * DK + GCOLS)
    f32 = np.float32
    bf16 = ml_dtypes.bfloat16
    c = np.ascontiguousarray
    return {
        "xqT": c(query[b].T.astype(bf16)),
        "xkT": c(key_[b].T.astype(bf16)),
        "xvT": c(value[b].T.astype(bf16)),
        "wq": c(Wq[:, cs].astype(bf16)),
        "wk": c(Wk[:, cs].astype(bf16)),
        "wv": c(Wv[:, cs].astype(bf16)),
        "wo": c(Wo[cs, :].astype(bf16)),
        "bq": c(bq[cs].astype(f32)),
        "bk": c(bk[cs].astype(f32)),
        "bv": c(bv[cs].astype(bf16)[None, :]),
        "wgq": c((Wg[:D, H0 : H0 + HPC] / S).astype(f32)),
        "wgk": c((Wg[D:, H0 : H0 + HPC] / S).astype(f32)),
        "bg": c(bg[H0 : H0 + HPC].astype(f32)[None, :]),
        "mtri": np.triu(np.ones((P, P), bf16)),
    }


_last_results = None


def kernel(query, key_, value, mask, Wq, bq, Wk, bk, Wv, bv, Wo, bo, Wg, bg):
    global _last_results
    from concourse.bass_utils import run_bass_kernel_spmd

    query = np.asarray(query)
    key_ = np.asarray(key_)
    value = np.asarray(value)
    mask = np.asarray(mask)
    B, S, D = query.shape
    H = np.asarray(bg).shape[0]
    DK = D // H
    DOUT = np.asarray(Wo).shape[1]
    NC_ = 8
    GROUPS = NC_ // B
    HPC = H // GROUPS

    causal = bool(
        np.array_equal(mask[0, 0], np.tril(np.ones((S, S), bool)))
    )
    if not causal:
        assert mask.all(), "only causal or all-true masks supported"

    key = (S, D, DOUT, HPC, DK, causal)
    if key not in _BUILD_CACHE:
        _BUILD_CACHE[key] = _build(*key)
    nc = _BUILD_CACHE[key]

    in_maps = []
    for c in range(NC_):
        b, gidx = divmod(c, GROUPS)
        in_maps.append(_prep_core_inputs(
            query, key_, value, Wq, bq, Wk, bk, Wv, bv, Wg, bg, Wo,
            b, gidx, S, D, HPC, DK))

    res = run_bass_kernel_spmd(nc, in_maps, core_ids=list(range(NC_)))
    _last_results = res

    out = np.zeros((B, S, DOUT), np.float32)
    for c in range(NC_):
        b = c // GROUPS
        out[b] += res.results[c]["out"]
    out += np.asarray(bo).astype(np.float32)
    return out
